# revision 4
# baseline (speedup 1.0000x reference)
"""Trainium2 Bass kernel for nn_ConvModule (dense_cnn).

Data-parallel over batch: 16 batch elems -> 8 cores x 2.
All matmuls in float32r (1 cycle/row, ~1.3e-4 rel rounding).
Pipeline per batch: transpose x -> dcnn conv (K=5) -> fused LN(C,S)+LN(C)
-> pointwise w1 + Silu -> conv2 (K=5) + GLU -> BN (sync via AllReduce)
-> pointwise w3 -> out (B,S,C).
"""
import sys
import numpy as np
from contextlib import ExitStack

sys.path.insert(0, "/opt/trn_rl_repo")

import concourse.bass as bass
import concourse.tile as tile
from concourse import bacc, mybir
from concourse.bass_utils import run_bass_kernel_spmd

F32 = mybir.dt.float32
F32R = mybir.dt.float32r
AF = mybir.ActivationFunctionType
OP = mybir.AluOpType

B, S, C, K = 16, 2048, 512, 5
NCORES = 8
BL = B // NCORES          # 2 batch elems per core
SW = 512                  # s-tile width
ST = S // SW              # 4 s-tiles
CT = C // 128             # 4 c-tiles
C2T = 2 * C // 128        # 8 2c-tiles
SP = S + 4                # padded s width (halo 2 each side)
EPS = 1e-5

LAST_RESULT = None
_NC = None


def _build():
    nc = bacc.Bacc("TRN2", target_bir_lowering=False, debug=False,
                   num_devices=NCORES)

    x_d = nc.dram_tensor("x", [BL, S, C], F32, kind="ExternalInput").ap()
    wa_d = nc.dram_tensor("wa", [128, K, CT, CT, 128], F32,
                          kind="ExternalInput").ap()
    w1_d = nc.dram_tensor("w1t", [128, CT, C2T, 128], F32,
                          kind="ExternalInput").ap()
    w2_d = nc.dram_tensor("w2t", [128, K, C2T, C2T, 128], F32,
                          kind="ExternalInput").ap()
    w3_d = nc.dram_tensor("w3t", [128, CT, 512], F32,
                          kind="ExternalInput").ap()
    dcnnb_d = nc.dram_tensor("dcnnb", [128, CT], F32, kind="ExternalInput").ap()
    b1_d = nc.dram_tensor("b1p", [128, C2T], F32, kind="ExternalInput").ap()
    b2_d = nc.dram_tensor("b2p", [128, C2T], F32, kind="ExternalInput").ap()
    b3_d = nc.dram_tensor("b3p", [1, 512], F32, kind="ExternalInput").ap()
    id_d = nc.dram_tensor("ident", [128, 128], F32, kind="ExternalInput").ap()
    out_d = nc.dram_tensor("out", [BL, S, C], F32, kind="ExternalOutput").ap()

    # internal DRAM
    bn_in = nc.dram_tensor("bn_in", [CT, 128, 2], F32)
    bn_out = nc.dram_tensor("bn_out", [CT, 128, 2], F32)

    with tile.TileContext(nc) as tc, ExitStack() as ctx:
        cpool = ctx.enter_context(tc.tile_pool(name="const", bufs=1))
        dscr = ctx.enter_context(tc.tile_pool(name="dram", bufs=1, space="DRAM"))
        pp_main = ctx.enter_context(tc.tile_pool(name="ppm", bufs=4, space="PSUM"))
        pp_misc = ctx.enter_context(tc.tile_pool(name="ppx", bufs=2, space="PSUM"))
        pp_stat = ctx.enter_context(tc.tile_pool(name="pps", bufs=1, space="PSUM"))

        h2scr = dscr.tile([BL, CT, 128, S], F32)

        # ---- constants ----
        wa_s = cpool.tile([128, K, CT, CT, 128], F32R)
        nc.sync.dma_start(out=wa_s, in_=wa_d.bitcast(F32R))
        dcnnb_s = cpool.tile([128, CT], F32)
        nc.sync.dma_start(out=dcnnb_s, in_=dcnnb_d)
        b1_s = cpool.tile([128, C2T], F32)
        nc.sync.dma_start(out=b1_s, in_=b1_d)
        b2_s = cpool.tile([128, C2T], F32)
        nc.sync.dma_start(out=b2_s, in_=b2_d)
        id_s = cpool.tile([128, 128], F32)
        nc.sync.dma_start(out=id_s, in_=id_d)
        zeros_s = cpool.tile([128, C2T, 2], F32)
        nc.vector.memset(zeros_s, 0.0)
        ones1_f = cpool.tile([1, 128], F32)
        nc.vector.memset(ones1_f, 1.0)
        ones1_s = cpool.tile([1, 128], F32R)
        nc.scalar.activation(ones1_s, ones1_f, AF.Copy)
        onesc_f = cpool.tile([128, 1], F32)
        nc.vector.memset(onesc_f, 1.0)
        onesc_s = cpool.tile([128, 1], F32R)
        nc.scalar.activation(onesc_s, onesc_f, AF.Copy)
        eps1_s = cpool.tile([1, 1], F32)
        nc.vector.memset(eps1_s, EPS)
        epsb_s = cpool.tile([128, 1], F32)
        nc.vector.memset(epsb_s, EPS)
        st6 = [cpool.tile([128, BL * ST, 6], F32, tag=f"st6_{j}",
                          name=f"st6_{j}")
               for j in range(CT)]
        bnpack_s = cpool.tile([128, CT, 2], F32)
        bnsum_s = cpool.tile([128, CT, 2], F32)
        mu_s = cpool.tile([128, CT], F32)
        rsb_s = cpool.tile([128, CT], F32)
        tmpb_s = cpool.tile([128, CT], F32)

        with tc.tile_pool(name="w1s", bufs=1) as w1pool, \
             tc.tile_pool(name="xnat", bufs=1) as xnpool, \
             tc.tile_pool(name="xh", bufs=1) as bigpool, \
             tc.tile_pool(name="ypl", bufs=1) as ypool, \
             tc.tile_pool(name="rows", bufs=1) as rpool, \
             tc.tile_pool(name="y2", bufs=1) as y2pool, \
             tc.tile_pool(name="ag", bufs=1) as agpool, \
             tc.tile_pool(name="h2o", bufs=2) as h2pool, \
             tc.tile_pool(name="w2s", bufs=1) as w2pool:

            for b in range(BL):
                # ---------- stage A: transpose x into padded (C,S) ----------
                with nc.named_scope(f"trans{b}"):
                    xpad = bigpool.tile([128, CT, SP], F32R, tag="xh")
                    nc.scalar.activation(xpad[:, :, 0:2], zeros_s[:, 0:CT, :],
                                         AF.Copy)
                    nc.scalar.activation(xpad[:, :, SP - 2:SP],
                                         zeros_s[:, 0:CT, :], AF.Copy)
                    for sb in range(S // 128):
                        xn = xnpool.tile([128, C], F32, tag="xn")
                        nc.sync.dma_start(out=xn,
                                          in_=x_d[b, sb * 128:(sb + 1) * 128, :])
                        for i in range(CT):
                            tp = pp_misc.tile([128, 128], F32, tag="tp")
                            nc.tensor.transpose(tp, xn[:, i * 128:(i + 1) * 128],
                                                id_s)
                            nc.scalar.activation(
                                xpad[:, i, 2 + sb * 128:2 + (sb + 1) * 128],
                                tp, AF.Copy)

                # ---------- stage A2: dcnn conv ----------
                with nc.named_scope(f"dcnn{b}"):
                    y = ypool.tile([128, CT, S], F32R, tag="y")
                    for j in range(CT):
                        for st in range(ST):
                            ps = pp_main.tile([128, SW], F32, tag="mm")
                            n = 0
                            for i in range(CT):
                                for k in range(K):
                                    nc.tensor.matmul(
                                        ps, wa_s[:, k, i, j, :],
                                        xpad[:, i, st * SW + k:st * SW + k + SW],
                                        start=(n == 0), stop=(n == CT * K - 1))
                                    n += 1
                            nc.scalar.activation(
                                y[:, j, st * SW:(st + 1) * SW], ps,
                                AF.Identity, bias=dcnnb_s[:, j:j + 1])

                # ---------- stage A3: LN stats ----------
                with nc.named_scope(f"stats{b}"):
                    mc_t = rpool.tile([1, ST, SW], F32R, tag="mc")
                    ex2_t = rpool.tile([1, ST, SW], F32R, tag="ex2")
                    scal = rpool.tile([1, 16], F32, tag="scal")
                    for st in range(ST):
                        pss = pp_stat.tile([1, SW], F32, tag="colsum")
                        psq = pp_stat.tile([1, SW], F32, tag="colsq")
                        for j in range(CT):
                            y2 = y2pool.tile([128, SW], F32R, tag="y2")
                            nc.scalar.activation(
                                y2, y[:, j, st * SW:(st + 1) * SW].bitcast(F32),
                                AF.Square)
                            nc.tensor.matmul(pss, onesc_s,
                                             y[:, j, st * SW:(st + 1) * SW],
                                             start=(j == 0), stop=(j == CT - 1))
                            nc.tensor.matmul(psq, onesc_s, y2,
                                             start=(j == 0), stop=(j == CT - 1))
                        # m_c = colsum/C ; ex2 = colsumsq/C
                        nc.vector.tensor_scalar_mul(mc_t[:, st, :], pss, 1.0 / C)
                        nc.vector.tensor_scalar_mul(ex2_t[:, st, :], psq, 1.0 / C)
                    # global stats
                    nc.vector.tensor_reduce(scal[:, 0:1], mc_t.bitcast(F32),
                                            mybir.AxisListType.XY, OP.add)
                    nc.vector.tensor_reduce(scal[:, 1:2], ex2_t.bitcast(F32),
                                            mybir.AxisListType.XY, OP.add)
                    # mu_g = scal0/S ; e2 = scal1/S
                    nc.vector.tensor_scalar_mul(scal[:, 2:3], scal[:, 0:1], 1.0 / S)
                    nc.vector.tensor_scalar_mul(scal[:, 3:4], scal[:, 1:2], 1.0 / S)
                    # var_g = e2 - mu^2
                    nc.vector.tensor_mul(scal[:, 4:5], scal[:, 2:3], scal[:, 2:3])
                    nc.vector.tensor_sub(scal[:, 5:6], scal[:, 3:4], scal[:, 4:5])
                    # rs = exp(-0.5 ln(var+eps)) ; rs2 = rs^2
                    nc.scalar.activation(scal[:, 6:7], scal[:, 5:6], AF.Ln,
                                         bias=eps1_s)
                    nc.scalar.activation(scal[:, 7:8], scal[:, 6:7], AF.Exp,
                                         scale=-0.5)
                    nc.vector.tensor_mul(scal[:, 8:9], scal[:, 7:8], scal[:, 7:8])

                # ---------- stage A4: LN transform rows + apply ----------
                with nc.named_scope(f"lnapply{b}"):
                    for st in range(ST):
                        rtmp = h2pool.tile([128, SW], F32, tag="h2")
                        # tmp = m_c^2 ; v = ex2 - tmp ; v = v*rs2 + eps
                        nc.vector.tensor_mul(rtmp[0:1, :],
                                             mc_t[:, st, :].bitcast(F32),
                                             mc_t[:, st, :].bitcast(F32))
                        nc.vector.tensor_sub(ex2_t[:, st, :],
                                             ex2_t[:, st, :].bitcast(F32),
                                             rtmp[0:1, :])
                        nc.vector.tensor_scalar(ex2_t[:, st, :],
                                                ex2_t[:, st, :].bitcast(F32),
                                                scal[:, 8:9], EPS,
                                                OP.mult, OP.add)
                        # A = rs * exp(-0.5 ln(v))  (stored in ex2 slot)
                        nc.scalar.activation(rtmp[0:1, :],
                                             ex2_t[:, st, :].bitcast(F32),
                                             AF.Ln, bias=0.0)
                        nc.scalar.activation(ex2_t[:, st, :], rtmp[0:1, :],
                                             AF.Exp, scale=-0.5)
                        nc.vector.tensor_scalar_mul(ex2_t[:, st, :],
                                                    ex2_t[:, st, :].bitcast(F32),
                                                    scal[:, 7:8])
                        # B = m_c * A (stored in mc slot)
                        nc.vector.tensor_mul(mc_t[:, st, :],
                                             mc_t[:, st, :].bitcast(F32),
                                             ex2_t[:, st, :].bitcast(F32))
                        # broadcast A,B to 128 partitions
                        pa = pp_misc.tile([128, SW], F32, tag="tp")
                        pb = pp_misc.tile([128, SW], F32, tag="tp")
                        nc.tensor.matmul(pa, ones1_s, ex2_t[:, st, :],
                                         start=True, stop=True)
                        nc.tensor.matmul(pb, ones1_s, mc_t[:, st, :],
                                         start=True, stop=True)
                        for j in range(CT):
                            sl = y[:, j, st * SW:(st + 1) * SW]
                            nc.vector.tensor_mul(sl, sl.bitcast(F32), pa)
                            nc.vector.tensor_sub(sl, sl.bitcast(F32), pb)

                # ---------- stage B: w1 + Silu ----------
                with nc.named_scope(f"w1{b}"):
                    h = bigpool.tile([128, C2T, SP], F32R, tag="xh")
                    nc.scalar.activation(h[:, :, 0:2], zeros_s, AF.Copy)
                    nc.scalar.activation(h[:, :, SP - 2:SP], zeros_s, AF.Copy)
                    for j2 in range(C2T):
                        w1t = w1pool.tile([128, CT, 128], F32R, tag="w1")
                        nc.sync.dma_start(out=w1t,
                                          in_=w1_d[:, :, j2, :].bitcast(F32R))
                        for st in range(ST):
                            ps = pp_main.tile([128, SW], F32, tag="mm")
                            for i in range(CT):
                                nc.tensor.matmul(
                                    ps, w1t[:, i, :],
                                    y[:, i, st * SW:(st + 1) * SW],
                                    start=(i == 0), stop=(i == CT - 1))
                            nc.scalar.activation(
                                h[:, j2, 2 + st * SW:2 + (st + 1) * SW], ps,
                                AF.Silu, bias=b1_s[:, j2:j2 + 1])

                # ---------- stage C: conv2 + GLU + BN stats ----------
                with nc.named_scope(f"conv2_{b}"):
                    for j2 in range(CT):
                        w2a = w2pool.tile([128, K, C2T, 128], F32R, tag="w2a")
                        nc.sync.dma_start(out=w2a,
                                          in_=w2_d[:, :, :, j2, :].bitcast(F32R))
                        w2g = w2pool.tile([128, K, C2T, 128], F32R, tag="w2g")
                        nc.sync.dma_start(
                            out=w2g, in_=w2_d[:, :, :, j2 + CT, :].bitcast(F32R))
                        for st in range(ST):
                            psa = pp_main.tile([128, SW], F32, tag="mm")
                            n = 0
                            for i2 in range(C2T):
                                for k in range(K):
                                    nc.tensor.matmul(
                                        psa, w2a[:, k, i2, :],
                                        h[:, i2, st * SW + k:st * SW + k + SW],
                                        start=(n == 0),
                                        stop=(n == C2T * K - 1))
                                    n += 1
                            psg = pp_main.tile([128, SW], F32, tag="mm")
                            n = 0
                            for i2 in range(C2T):
                                for k in range(K):
                                    nc.tensor.matmul(
                                        psg, w2g[:, k, i2, :],
                                        h[:, i2, st * SW + k:st * SW + k + SW],
                                        start=(n == 0),
                                        stop=(n == C2T * K - 1))
                                    n += 1
                            g_sb = agpool.tile([128, SW], F32, tag="g")
                            nc.scalar.activation(g_sb, psg, AF.Sigmoid,
                                                 bias=b2_s[:, j2 + CT:j2 + CT + 1])
                            h2t = h2pool.tile([128, SW], F32, tag="h2")
                            nc.vector.tensor_scalar(h2t, psa,
                                                    b2_s[:, j2:j2 + 1], None,
                                                    OP.add)
                            nc.vector.tensor_mul(h2t, h2t, g_sb)
                            nc.vector.bn_stats(st6[j2][:, b * ST + st, :], h2t)
                            nc.sync.dma_start(
                                out=h2scr[b, j2, :, st * SW:(st + 1) * SW],
                                in_=h2t)

            # ---------- BN finalize + collective ----------
            with nc.named_scope("bnsync"):
                for j in range(CT):
                    mv = h2pool.tile([128, SW], F32, tag="h2")
                    nc.vector.bn_aggr(mv[:, 0:2], st6[j])
                    # sum = mean*N ; sumsq = (var+mean^2)*N   (N = BL*S)
                    nc.vector.tensor_mul(mv[:, 2:3], mv[:, 0:1], mv[:, 0:1])
                    nc.vector.tensor_add(mv[:, 3:4], mv[:, 1:2], mv[:, 2:3])
                    nc.vector.tensor_scalar_mul(bnpack_s[:, j, 0:1], mv[:, 0:1],
                                                float(BL * S))
                    nc.vector.tensor_scalar_mul(bnpack_s[:, j, 1:2], mv[:, 3:4],
                                                float(BL * S))
                    nc.sync.dma_start(out=bn_in.ap()[j], in_=bnpack_s[:, j, :])
                cc = nc.gpsimd.collective_compute(
                    "AllReduce", OP.add,
                    replica_groups=[list(range(NCORES))],
                    ins=[bn_in.ap()], outs=[bn_out.ap()])
                for j in range(CT):
                    rd = nc.sync.dma_start(out=bnsum_s[:, j, :],
                                           in_=bn_out.ap()[j])
                    tile.add_dep_helper(rd.ins, cc.ins, sync=True,
                                        reason="bn allreduce->read")

        # ---------- stage D: BN apply + w3 ----------
        with tc.tile_pool(name="h2r", bufs=2 * CT + 1) as h2rpool, \
             tc.tile_pool(name="dconst", bufs=1) as dcpool, \
             tc.tile_pool(name="ost", bufs=3) as opool:
            with nc.named_scope("bnmath"):
                nc.vector.tensor_scalar_mul(mu_s, bnsum_s[:, :, 0],
                                            1.0 / (B * S))
                nc.vector.tensor_scalar_mul(rsb_s, bnsum_s[:, :, 1],
                                            1.0 / (B * S))
                nc.vector.tensor_mul(tmpb_s, mu_s, mu_s)
                nc.vector.tensor_sub(rsb_s, rsb_s, tmpb_s)
                nc.scalar.activation(tmpb_s, rsb_s, AF.Ln, bias=epsb_s)
                nc.scalar.activation(rsb_s, tmpb_s, AF.Exp, scale=-0.5)

            w3t = dcpool.tile([128, CT, 512], F32R)
            nc.sync.dma_start(out=w3t, in_=w3_d.bitcast(F32R))
            b3b = dcpool.tile([128, 512], F32)
            nc.sync.dma_start(out=b3b, in_=b3_d.partition_broadcast(128))

            with nc.named_scope("w3"):
                for b in range(BL):
                    h2r = []
                    for i in range(CT):
                        t = h2rpool.tile([128, S], F32R, tag="h2r",
                                         name=f"h2r_{b}_{i}")
                        nc.sync.dma_start(out=t, in_=h2scr[b, i].bitcast(F32R))
                        nc.vector.tensor_scalar(t, t.bitcast(F32),
                                                mu_s[:, i:i + 1],
                                                rsb_s[:, i:i + 1],
                                                OP.subtract, OP.mult)
                        h2r.append(t)
                    for sb in range(S // 128):
                        ps = pp_main.tile([128, SW], F32, tag="mm")
                        for i in range(CT):
                            nc.tensor.matmul(ps, h2r[i][:, sb * 128:(sb + 1) * 128],
                                             w3t[:, i, :],
                                             start=(i == 0), stop=(i == CT - 1))
                        ot = opool.tile([128, 512], F32, tag="o")
                        nc.vector.tensor_add(ot, ps, b3b)
                        nc.sync.dma_start(
                            out=out_d[b, sb * 128:(sb + 1) * 128, :], in_=ot)

    nc.compile()
    return nc


def _prep(inputs):
    x = np.ascontiguousarray(np.asarray(inputs["x"], np.float32))
    dcnn_w = np.asarray(inputs["dcnn_w"], np.float32)
    dcnn_b = np.asarray(inputs["dcnn_b"], np.float32)
    ln_g = np.asarray(inputs["ln_g"], np.float32)
    ln_b = np.asarray(inputs["ln_b"], np.float32)
    w1 = np.asarray(inputs["w1"], np.float32)[:, :, 0]
    b1 = np.asarray(inputs["b1"], np.float32)
    w2 = np.asarray(inputs["w2"], np.float32)
    b2 = np.asarray(inputs["b2"], np.float32)
    bn_g = np.asarray(inputs["bn_g"], np.float32)
    bn_b = np.asarray(inputs["bn_b"], np.float32)
    w3 = np.asarray(inputs["w3"], np.float32)[:, :, 0]
    b3 = np.asarray(inputs["b3"], np.float32)

    # dcnn lhsT pack: (ci, k, i, j, co)
    t = dcnn_w.reshape(CT, 128, CT, 128, K)
    wa = np.ascontiguousarray(t.transpose(3, 4, 2, 0, 1))
    # fold ln affine into w1/b1
    w1f = w1 * ln_g[None, :]
    b1p = b1 + w1 @ ln_b
    t = w1f.reshape(C2T, 128, CT, 128)
    w1t = np.ascontiguousarray(t.transpose(3, 2, 0, 1))
    # w2 pack
    t = w2.reshape(C2T, 128, C2T, 128, K)
    w2t = np.ascontiguousarray(t.transpose(3, 4, 2, 0, 1))
    # fold bn affine into w3/b3
    w3f = w3 * bn_g[None, :]
    b3p = b3 + w3 @ bn_b
    t = w3f.reshape(512, CT, 128)
    w3t = np.ascontiguousarray(t.transpose(2, 1, 0))

    common = {
        "wa": wa,
        "w1t": w1t,
        "w2t": w2t,
        "w3t": w3t,
        "dcnnb": np.ascontiguousarray(dcnn_b.reshape(CT, 128).T),
        "b1p": np.ascontiguousarray(b1p.reshape(C2T, 128).T),
        "b2p": np.ascontiguousarray(b2.reshape(C2T, 128).T),
        "b3p": np.ascontiguousarray(b3p.reshape(1, 512)),
        "ident": np.eye(128, dtype=np.float32),
    }
    in_maps = []
    for c in range(NCORES):
        m = dict(common)
        m["x"] = np.ascontiguousarray(x[c * BL:(c + 1) * BL])
        in_maps.append(m)
    return in_maps


def kernel(**inputs) -> np.ndarray:
    global LAST_RESULT, _NC
    if _NC is None:
        _NC = _build()
    in_maps = _prep(inputs)
    res = run_bass_kernel_spmd(_NC, in_maps, list(range(NCORES)))
    LAST_RESULT = res
    out = np.empty((B, S, C), np.float32)
    for c in range(NCORES):
        out[c * BL:(c + 1) * BL] = res.results[c]["out"]
    return out


# revision 5
# speedup vs baseline: 1.0890x; 1.0890x over previous
"""Trainium2 Bass kernel for nn_ConvModule (dense_cnn).

Data-parallel over batch: 16 batch elems -> 8 cores x 2.
All matmuls in float32r (1 cycle/row, ~1.3e-4 rel rounding).
Pipeline per batch: transpose x -> dcnn conv (K=5) -> fused LN(C,S)+LN(C)
-> pointwise w1 + Silu -> conv2 (K=5) + GLU -> BN (sync via AllReduce)
-> pointwise w3 -> out (B,S,C).
"""
import sys
import numpy as np
from contextlib import ExitStack

sys.path.insert(0, "/opt/trn_rl_repo")

import concourse.bass as bass
import concourse.tile as tile
from concourse import bacc, mybir
from concourse.bass_utils import run_bass_kernel_spmd

F32 = mybir.dt.float32
F32R = mybir.dt.float32r
BF16 = mybir.dt.bfloat16
AF = mybir.ActivationFunctionType
OP = mybir.AluOpType

B, S, C, K = 16, 2048, 512, 5
NCORES = 8
BL = B // NCORES          # 2 batch elems per core
SW = 512                  # s-tile width
ST = S // SW              # 4 s-tiles
CT = C // 128             # 4 c-tiles
C2T = 2 * C // 128        # 8 2c-tiles
SP = S + 4                # padded s width (halo 2 each side)
EPS = 1e-5

LAST_RESULT = None
_NC = None


def _build():
    nc = bacc.Bacc("TRN2", target_bir_lowering=False, debug=False,
                   num_devices=NCORES)

    x_d = nc.dram_tensor("x", [BL, S, C], F32, kind="ExternalInput").ap()
    wa_d = nc.dram_tensor("wa", [128, K, CT, CT, 128], BF16,
                          kind="ExternalInput").ap()
    w1_d = nc.dram_tensor("w1t", [128, CT, C2T, 128], BF16,
                          kind="ExternalInput").ap()
    w2_d = nc.dram_tensor("w2t", [128, K, C2T, C2T, 128], BF16,
                          kind="ExternalInput").ap()
    w3_d = nc.dram_tensor("w3t", [128, CT, 512], BF16,
                          kind="ExternalInput").ap()
    dcnnb_d = nc.dram_tensor("dcnnb", [128, CT], F32, kind="ExternalInput").ap()
    b1_d = nc.dram_tensor("b1p", [128, C2T], F32, kind="ExternalInput").ap()
    b2_d = nc.dram_tensor("b2p", [128, C2T], F32, kind="ExternalInput").ap()
    b3_d = nc.dram_tensor("b3p", [1, 512], F32, kind="ExternalInput").ap()
    id_d = nc.dram_tensor("ident", [128, 128], F32, kind="ExternalInput").ap()
    out_d = nc.dram_tensor("out", [BL, S, C], F32, kind="ExternalOutput").ap()

    # internal DRAM
    bn_in = nc.dram_tensor("bn_in", [CT, 128, 2], F32)
    bn_out = nc.dram_tensor("bn_out", [CT, 128, 2], F32)

    with tile.TileContext(nc) as tc, ExitStack() as ctx:
        cpool = ctx.enter_context(tc.tile_pool(name="const", bufs=1))
        dscr = ctx.enter_context(tc.tile_pool(name="dram", bufs=1, space="DRAM"))
        pp_main = ctx.enter_context(tc.tile_pool(name="ppm", bufs=4, space="PSUM"))
        pp_misc = ctx.enter_context(tc.tile_pool(name="ppx", bufs=2, space="PSUM"))
        pp_stat = ctx.enter_context(tc.tile_pool(name="pps", bufs=1, space="PSUM"))

        h2scr = dscr.tile([BL, CT, 128, S], BF16)

        # ---- constants ----
        wa_s = cpool.tile([128, K, CT, CT, 128], BF16)
        nc.sync.dma_start(out=wa_s, in_=wa_d)
        dcnnb_s = cpool.tile([128, CT], F32)
        nc.sync.dma_start(out=dcnnb_s, in_=dcnnb_d)
        b1_s = cpool.tile([128, C2T], F32)
        nc.sync.dma_start(out=b1_s, in_=b1_d)
        b2_s = cpool.tile([128, C2T], F32)
        nc.sync.dma_start(out=b2_s, in_=b2_d)
        id_s = cpool.tile([128, 128], F32)
        nc.sync.dma_start(out=id_s, in_=id_d)
        zeros_s = cpool.tile([128, C2T, 2], F32)
        nc.vector.memset(zeros_s, 0.0)
        ones1_f = cpool.tile([1, 128], F32)
        nc.vector.memset(ones1_f, 1.0)
        ones1_s = cpool.tile([1, 128], F32R)
        nc.scalar.activation(ones1_s, ones1_f, AF.Copy)
        onesc_f = cpool.tile([128, 1], F32)
        nc.vector.memset(onesc_f, 1.0)
        onesc_s = cpool.tile([128, 1], BF16)
        nc.scalar.activation(onesc_s, onesc_f, AF.Copy)
        eps1_s = cpool.tile([1, 1], F32)
        nc.vector.memset(eps1_s, EPS)
        epsb_s = cpool.tile([128, 1], F32)
        nc.vector.memset(epsb_s, EPS)
        st6 = [cpool.tile([128, BL * ST, 6], F32, tag=f"st6_{j}",
                          name=f"st6_{j}")
               for j in range(CT)]
        bnpack_s = cpool.tile([128, CT, 2], F32)
        bnsum_s = cpool.tile([128, CT, 2], F32)
        mu_s = cpool.tile([128, CT], F32)
        rsb_s = cpool.tile([128, CT], F32)
        tmpb_s = cpool.tile([128, CT], F32)

        with tc.tile_pool(name="w1s", bufs=1) as w1pool, \
             tc.tile_pool(name="xnat", bufs=1) as xnpool, \
             tc.tile_pool(name="xh", bufs=1) as bigpool, \
             tc.tile_pool(name="ypl", bufs=1) as ypool, \
             tc.tile_pool(name="rows", bufs=1) as rpool, \
             tc.tile_pool(name="y2", bufs=1) as y2pool, \
             tc.tile_pool(name="ag", bufs=1) as agpool, \
             tc.tile_pool(name="h2o", bufs=2) as h2pool, \
             tc.tile_pool(name="w2s", bufs=2) as w2pool:

            for b in range(BL):
                # ---------- stage A: transpose x into padded (C,S) ----------
                with nc.named_scope(f"trans{b}"):
                    xpad = bigpool.tile([128, CT, SP], BF16, tag="xh")
                    nc.scalar.activation(xpad[:, :, 0:2], zeros_s[:, 0:CT, :],
                                         AF.Copy)
                    nc.scalar.activation(xpad[:, :, SP - 2:SP],
                                         zeros_s[:, 0:CT, :], AF.Copy)
                    for sb in range(S // 128):
                        xn = xnpool.tile([128, C], F32, tag="xn")
                        nc.sync.dma_start(out=xn,
                                          in_=x_d[b, sb * 128:(sb + 1) * 128, :])
                        for i in range(CT):
                            tp = pp_misc.tile([128, 128], F32, tag="tp")
                            nc.tensor.transpose(tp, xn[:, i * 128:(i + 1) * 128],
                                                id_s)
                            nc.scalar.activation(
                                xpad[:, i, 2 + sb * 128:2 + (sb + 1) * 128],
                                tp, AF.Copy)

                # ---------- stage A2: dcnn conv ----------
                with nc.named_scope(f"dcnn{b}"):
                    y = ypool.tile([128, CT, S], BF16, tag="y")
                    for j in range(CT):
                        for st in range(ST):
                            ps = pp_main.tile([128, SW], F32, tag="mm")
                            n = 0
                            for i in range(CT):
                                for k in range(K):
                                    nc.tensor.matmul(
                                        ps, wa_s[:, k, i, j, :],
                                        xpad[:, i, st * SW + k:st * SW + k + SW],
                                        start=(n == 0), stop=(n == CT * K - 1))
                                    n += 1
                            nc.scalar.activation(
                                y[:, j, st * SW:(st + 1) * SW], ps,
                                AF.Identity, bias=dcnnb_s[:, j:j + 1])

                # ---------- stage A3: LN stats ----------
                with nc.named_scope(f"stats{b}"):
                    mc_t = rpool.tile([1, ST, SW], F32R, tag="mc")
                    ex2_t = rpool.tile([1, ST, SW], F32R, tag="ex2")
                    scal = rpool.tile([1, 16], F32, tag="scal")
                    for st in range(ST):
                        pss = pp_stat.tile([1, SW], F32, tag="colsum")
                        psq = pp_stat.tile([1, SW], F32, tag="colsq")
                        for j in range(CT):
                            y2 = y2pool.tile([128, SW], BF16, tag="y2")
                            nc.scalar.activation(
                                y2, y[:, j, st * SW:(st + 1) * SW],
                                AF.Square)
                            nc.tensor.matmul(pss, onesc_s,
                                             y[:, j, st * SW:(st + 1) * SW],
                                             start=(j == 0), stop=(j == CT - 1))
                            nc.tensor.matmul(psq, onesc_s, y2,
                                             start=(j == 0), stop=(j == CT - 1))
                        # m_c = colsum/C ; ex2 = colsumsq/C
                        nc.vector.tensor_scalar_mul(mc_t[:, st, :], pss, 1.0 / C)
                        nc.vector.tensor_scalar_mul(ex2_t[:, st, :], psq, 1.0 / C)
                    # global stats
                    nc.vector.tensor_reduce(scal[:, 0:1], mc_t.bitcast(F32),
                                            mybir.AxisListType.XY, OP.add)
                    nc.vector.tensor_reduce(scal[:, 1:2], ex2_t.bitcast(F32),
                                            mybir.AxisListType.XY, OP.add)
                    # mu_g = scal0/S ; e2 = scal1/S
                    nc.vector.tensor_scalar_mul(scal[:, 2:3], scal[:, 0:1], 1.0 / S)
                    nc.vector.tensor_scalar_mul(scal[:, 3:4], scal[:, 1:2], 1.0 / S)
                    # var_g = e2 - mu^2
                    nc.vector.tensor_mul(scal[:, 4:5], scal[:, 2:3], scal[:, 2:3])
                    nc.vector.tensor_sub(scal[:, 5:6], scal[:, 3:4], scal[:, 4:5])
                    # rs = exp(-0.5 ln(var+eps)) ; rs2 = rs^2
                    nc.scalar.activation(scal[:, 6:7], scal[:, 5:6], AF.Ln,
                                         bias=eps1_s)
                    nc.scalar.activation(scal[:, 7:8], scal[:, 6:7], AF.Exp,
                                         scale=-0.5)
                    nc.vector.tensor_mul(scal[:, 8:9], scal[:, 7:8], scal[:, 7:8])

                # ---------- stage A4: LN transform rows + apply ----------
                with nc.named_scope(f"lnapply{b}"):
                    for st in range(ST):
                        rtmp = h2pool.tile([128, SW], F32, tag="h2")
                        # tmp = m_c^2 ; v = ex2 - tmp ; v = v*rs2 + eps
                        nc.vector.tensor_mul(rtmp[0:1, :],
                                             mc_t[:, st, :].bitcast(F32),
                                             mc_t[:, st, :].bitcast(F32))
                        nc.vector.tensor_sub(ex2_t[:, st, :],
                                             ex2_t[:, st, :].bitcast(F32),
                                             rtmp[0:1, :])
                        nc.vector.tensor_scalar(ex2_t[:, st, :],
                                                ex2_t[:, st, :].bitcast(F32),
                                                scal[:, 8:9], EPS,
                                                OP.mult, OP.add)
                        # A = rs * exp(-0.5 ln(v))  (stored in ex2 slot)
                        nc.scalar.activation(rtmp[0:1, :],
                                             ex2_t[:, st, :].bitcast(F32),
                                             AF.Ln, bias=0.0)
                        nc.scalar.activation(ex2_t[:, st, :], rtmp[0:1, :],
                                             AF.Exp, scale=-0.5)
                        nc.vector.tensor_scalar_mul(ex2_t[:, st, :],
                                                    ex2_t[:, st, :].bitcast(F32),
                                                    scal[:, 7:8])
                        # B = m_c * A (stored in mc slot)
                        nc.vector.tensor_mul(mc_t[:, st, :],
                                             mc_t[:, st, :].bitcast(F32),
                                             ex2_t[:, st, :].bitcast(F32))
                        # broadcast A,B to 128 partitions
                        pa = pp_misc.tile([128, SW], F32, tag="tp")
                        pb = pp_misc.tile([128, SW], F32, tag="tp")
                        nc.tensor.matmul(pa, ones1_s, ex2_t[:, st, :],
                                         start=True, stop=True)
                        nc.tensor.matmul(pb, ones1_s, mc_t[:, st, :],
                                         start=True, stop=True)
                        ab = agpool.tile([128, SW], BF16, tag="ab")
                        nc.scalar.activation(ab, pa, AF.Copy)
                        bb = agpool.tile([128, SW], BF16, tag="bb")
                        nc.scalar.activation(bb, pb, AF.Copy)
                        for j in range(CT):
                            sl = y[:, j, st * SW:(st + 1) * SW]
                            nc.vector.tensor_mul(sl, sl, ab)
                            nc.vector.tensor_sub(sl, sl, bb)

                # ---------- stage B: w1 + Silu ----------
                with nc.named_scope(f"w1{b}"):
                    h = bigpool.tile([128, C2T, SP], BF16, tag="xh")
                    nc.scalar.activation(h[:, :, 0:2], zeros_s, AF.Copy)
                    nc.scalar.activation(h[:, :, SP - 2:SP], zeros_s, AF.Copy)
                    for j2 in range(C2T):
                        w1t = w1pool.tile([128, CT, 128], BF16, tag="w1")
                        nc.sync.dma_start(out=w1t,
                                          in_=w1_d[:, :, j2, :])
                        for st in range(ST):
                            ps = pp_main.tile([128, SW], F32, tag="mm")
                            for i in range(CT):
                                nc.tensor.matmul(
                                    ps, w1t[:, i, :],
                                    y[:, i, st * SW:(st + 1) * SW],
                                    start=(i == 0), stop=(i == CT - 1))
                            nc.scalar.activation(
                                h[:, j2, 2 + st * SW:2 + (st + 1) * SW], ps,
                                AF.Silu, bias=b1_s[:, j2:j2 + 1])

                # ---------- stage C: conv2 + GLU + BN stats ----------
                with nc.named_scope(f"conv2_{b}"):
                    for j2 in range(CT):
                        w2a = w2pool.tile([128, K, C2T, 128], BF16, tag="w2a")
                        nc.sync.dma_start(out=w2a,
                                          in_=w2_d[:, :, :, j2, :])
                        w2g = w2pool.tile([128, K, C2T, 128], BF16, tag="w2g")
                        nc.sync.dma_start(
                            out=w2g, in_=w2_d[:, :, :, j2 + CT, :])
                        for st in range(ST):
                            psa = pp_main.tile([128, SW], F32, tag="mm")
                            n = 0
                            for i2 in range(C2T):
                                for k in range(K):
                                    nc.tensor.matmul(
                                        psa, w2a[:, k, i2, :],
                                        h[:, i2, st * SW + k:st * SW + k + SW],
                                        start=(n == 0),
                                        stop=(n == C2T * K - 1))
                                    n += 1
                            psg = pp_main.tile([128, SW], F32, tag="mm")
                            n = 0
                            for i2 in range(C2T):
                                for k in range(K):
                                    nc.tensor.matmul(
                                        psg, w2g[:, k, i2, :],
                                        h[:, i2, st * SW + k:st * SW + k + SW],
                                        start=(n == 0),
                                        stop=(n == C2T * K - 1))
                                    n += 1
                            g_sb = agpool.tile([128, SW], BF16, tag="g")
                            nc.scalar.activation(g_sb, psg, AF.Sigmoid,
                                                 bias=b2_s[:, j2 + CT:j2 + CT + 1])
                            h2t = h2pool.tile([128, SW], BF16, tag="h2")
                            nc.vector.tensor_scalar(h2t, psa,
                                                    b2_s[:, j2:j2 + 1], None,
                                                    OP.add)
                            nc.vector.tensor_mul(h2t, h2t, g_sb)
                            nc.vector.bn_stats(st6[j2][:, b * ST + st, :], h2t)
                            nc.sync.dma_start(
                                out=h2scr[b, j2, :, st * SW:(st + 1) * SW],
                                in_=h2t)

            # ---------- BN finalize + collective ----------
            with nc.named_scope("bnsync"):
                for j in range(CT):
                    mv = h2pool.tile([128, SW], F32, tag="h2")
                    nc.vector.bn_aggr(mv[:, 0:2], st6[j])
                    # sum = mean*N ; sumsq = (var+mean^2)*N   (N = BL*S)
                    nc.vector.tensor_mul(mv[:, 2:3], mv[:, 0:1], mv[:, 0:1])
                    nc.vector.tensor_add(mv[:, 3:4], mv[:, 1:2], mv[:, 2:3])
                    nc.vector.tensor_scalar_mul(bnpack_s[:, j, 0:1], mv[:, 0:1],
                                                float(BL * S))
                    nc.vector.tensor_scalar_mul(bnpack_s[:, j, 1:2], mv[:, 3:4],
                                                float(BL * S))
                    nc.sync.dma_start(out=bn_in.ap()[j], in_=bnpack_s[:, j, :])
                cc = nc.gpsimd.collective_compute(
                    "AllReduce", OP.add,
                    replica_groups=[list(range(NCORES))],
                    ins=[bn_in.ap()], outs=[bn_out.ap()])
                for j in range(CT):
                    rd = nc.sync.dma_start(out=bnsum_s[:, j, :],
                                           in_=bn_out.ap()[j])
                    tile.add_dep_helper(rd.ins, cc.ins, sync=True,
                                        reason="bn allreduce->read")

        # ---------- stage D: BN apply + w3 ----------
        with tc.tile_pool(name="h2r", bufs=2 * CT + 1) as h2rpool, \
             tc.tile_pool(name="dconst", bufs=1) as dcpool, \
             tc.tile_pool(name="ost", bufs=3) as opool:
            with nc.named_scope("bnmath"):
                nc.vector.tensor_scalar_mul(mu_s, bnsum_s[:, :, 0],
                                            1.0 / (B * S))
                nc.vector.tensor_scalar_mul(rsb_s, bnsum_s[:, :, 1],
                                            1.0 / (B * S))
                nc.vector.tensor_mul(tmpb_s, mu_s, mu_s)
                nc.vector.tensor_sub(rsb_s, rsb_s, tmpb_s)
                nc.scalar.activation(tmpb_s, rsb_s, AF.Ln, bias=epsb_s)
                nc.scalar.activation(rsb_s, tmpb_s, AF.Exp, scale=-0.5)

            w3t = dcpool.tile([128, CT, 512], BF16)
            nc.sync.dma_start(out=w3t, in_=w3_d)
            b3b = dcpool.tile([128, 512], F32)
            nc.sync.dma_start(out=b3b, in_=b3_d.partition_broadcast(128))

            with nc.named_scope("w3"):
                for b in range(BL):
                    h2r = []
                    for i in range(CT):
                        t = h2rpool.tile([128, S], BF16, tag="h2r",
                                         name=f"h2r_{b}_{i}")
                        nc.sync.dma_start(out=t, in_=h2scr[b, i])
                        nc.vector.tensor_scalar(t, t,
                                                mu_s[:, i:i + 1],
                                                rsb_s[:, i:i + 1],
                                                OP.subtract, OP.mult)
                        h2r.append(t)
                    for sb in range(S // 128):
                        ps = pp_main.tile([128, SW], F32, tag="mm")
                        for i in range(CT):
                            nc.tensor.matmul(ps, h2r[i][:, sb * 128:(sb + 1) * 128],
                                             w3t[:, i, :],
                                             start=(i == 0), stop=(i == CT - 1))
                        ot = opool.tile([128, 512], F32, tag="o")
                        nc.vector.tensor_add(ot, ps, b3b)
                        nc.sync.dma_start(
                            out=out_d[b, sb * 128:(sb + 1) * 128, :], in_=ot)

    nc.compile()
    return nc


def _prep(inputs):
    x = np.ascontiguousarray(np.asarray(inputs["x"], np.float32))
    dcnn_w = np.asarray(inputs["dcnn_w"], np.float32)
    dcnn_b = np.asarray(inputs["dcnn_b"], np.float32)
    ln_g = np.asarray(inputs["ln_g"], np.float32)
    ln_b = np.asarray(inputs["ln_b"], np.float32)
    w1 = np.asarray(inputs["w1"], np.float32)[:, :, 0]
    b1 = np.asarray(inputs["b1"], np.float32)
    w2 = np.asarray(inputs["w2"], np.float32)
    b2 = np.asarray(inputs["b2"], np.float32)
    bn_g = np.asarray(inputs["bn_g"], np.float32)
    bn_b = np.asarray(inputs["bn_b"], np.float32)
    w3 = np.asarray(inputs["w3"], np.float32)[:, :, 0]
    b3 = np.asarray(inputs["b3"], np.float32)

    # dcnn lhsT pack: (ci, k, i, j, co)
    t = dcnn_w.reshape(CT, 128, CT, 128, K)
    wa = np.ascontiguousarray(t.transpose(3, 4, 2, 0, 1))
    # fold ln affine into w1/b1
    w1f = w1 * ln_g[None, :]
    b1p = b1 + w1 @ ln_b
    t = w1f.reshape(C2T, 128, CT, 128)
    w1t = np.ascontiguousarray(t.transpose(3, 2, 0, 1))
    # w2 pack
    t = w2.reshape(C2T, 128, C2T, 128, K)
    w2t = np.ascontiguousarray(t.transpose(3, 4, 2, 0, 1))
    # fold bn affine into w3/b3
    w3f = w3 * bn_g[None, :]
    b3p = b3 + w3 @ bn_b
    t = w3f.reshape(512, CT, 128)
    w3t = np.ascontiguousarray(t.transpose(2, 1, 0))

    import ml_dtypes
    bf16 = ml_dtypes.bfloat16
    common = {
        "wa": wa.astype(bf16),
        "w1t": w1t.astype(bf16),
        "w2t": w2t.astype(bf16),
        "w3t": w3t.astype(bf16),
        "dcnnb": np.ascontiguousarray(dcnn_b.reshape(CT, 128).T),
        "b1p": np.ascontiguousarray(b1p.reshape(C2T, 128).T),
        "b2p": np.ascontiguousarray(b2.reshape(C2T, 128).T),
        "b3p": np.ascontiguousarray(b3p.reshape(1, 512)),
        "ident": np.eye(128, dtype=np.float32),
    }
    in_maps = []
    for c in range(NCORES):
        m = dict(common)
        m["x"] = np.ascontiguousarray(x[c * BL:(c + 1) * BL])
        in_maps.append(m)
    return in_maps


def kernel(**inputs) -> np.ndarray:
    global LAST_RESULT, _NC
    if _NC is None:
        _NC = _build()
    in_maps = _prep(inputs)
    res = run_bass_kernel_spmd(_NC, in_maps, list(range(NCORES)))
    LAST_RESULT = res
    out = np.empty((B, S, C), np.float32)
    for c in range(NCORES):
        out[c * BL:(c + 1) * BL] = res.results[c]["out"]
    return out
